# revision 13
# baseline (speedup 1.0000x reference)
"""MemoryBank scatter-gather kernel for 8 Trainium2 NeuronCores.

Reference (per token n of 2048, K=500 neighbor slots, len=lengths[n]):
    neigh = l2norm(wordmem[idx[n,:len]]); q = l2norm(word_embs[n])
    score = q @ neigh.T ; attn = softmax-over-valid(score)
    out   = attn @ bankmem[idx[n,:len]]

Design (v2 — minimize gathered rows; GpSimd desc-gen is ~8ns/row serial):
  * ONE combined bf16 table row per bank row: [w_hat(100) | pad | 1.0 at
    col 111 | bank(400)] = 512 bf16 = 1024B (256B-aligned for dma_gather).
    Each (token, neighbor) pair costs exactly one gathered row instead of
    separate wordmem + bankmem fetches: 64k rows/core vs 240k baseline.
  * Tokens length-sorted and snake-dealt into 16 groups of 128
    (core = g%8, slot = g//8) so per-core pair counts balance.  Within a
    (slot, bank) all pairs are packed token-major into 128-row gather
    columns; the program shape depends only on per-(slot,bank) column
    counts (max across cores, cached by signature).
  * Scores: host streams replicated q-hat rows (bf16, sequential DMA, no
    gather); DVE mult+reduce against the w part of the gathered rows.
  * attn accumulation on PE: per column j, lhsT[p,t] = (iota[t]==tokid[p])
    * exp(score[p]) built by one DVE tensor_scalar; matmul with
    rhs = gathered[:, j, 111:512] accumulates [128 tok, 1+400] in PSUM —
    column 0 (the table's constant 1.0) is the softmax denominator, so
    normalization is one reciprocal + scale at slot end.  Pad slots get
    tokid=-1 => lhsT row 0 => no contribution to doc or denom.
"""

import numpy as np
import ml_dtypes

import concourse.bacc as bacc
import concourse.mybir as mybir
import concourse.tile as tile
from concourse.bass_utils import run_bass_kernel_spmd

BF16 = ml_dtypes.bfloat16

NUM = 2048
K = 500
WD = 100
HD = 400
V = 100000
N_CORES = 8
NSLOT = 2                      # 2 groups of 128 tokens per core
NBANK = 4
BROWS = V // NBANK             # 25000 rows per bank (< int16 max)
TE = 512                       # combined row elems (bf16) = 1024B
ONE_POS = 111                  # constant 1.0 column (denominator trick)
RHS_W = 1 + HD                 # matmul rhs width: [1.0 | bank row]
QE = 128                       # replicated q-hat row elems (w part width)
CHUNK = 16                     # gather columns per instruction (2048 rows)

_CACHE: dict = {}


# --------------------------------------------------------------------------
# host planning
# --------------------------------------------------------------------------

def _wrap16(flat_i16):
    """i-th index -> (i%16, i//16), replicated to 128 partitions."""
    n = flat_i16.shape[0]
    assert n % 16 == 0
    blk = flat_i16.reshape(-1, 16).T.copy()            # [16, n/16]
    return np.tile(blk, (8, 1))                        # [128, n/16]


def _plan(idx, lengths):
    idx = np.asarray(idx, dtype=np.int64)
    lengths = np.asarray(lengths, dtype=np.int64)

    order = np.argsort(-lengths, kind="stable")
    g = order.reshape(K // 4 * 0 + NUM // 16, 16).copy()   # [128 rounds, 16]
    g[1::2] = g[1::2, ::-1]                                # snake deal
    groups = g.T.copy()                                    # [16, 128] token ids

    # per (group, bank): token-major packed local indices + owning position
    seg_loc = [[None] * NBANK for _ in range(16)]
    seg_pos = [[None] * NBANK for _ in range(16)]
    cnt = np.zeros((16, NBANK), dtype=np.int64)
    for j in range(16):
        locs = [[] for _ in range(NBANK)]
        poss = [[] for _ in range(NBANK)]
        for p in range(128):
            t = groups[j][p]
            v = idx[t, : lengths[t]]
            b = v // BROWS
            for bb in range(NBANK):
                lv = v[b == bb] - bb * BROWS
                locs[bb].append(lv)
                poss[bb].append(np.full(lv.shape[0], p, dtype=np.int64))
        for bb in range(NBANK):
            seg_loc[j][bb] = np.concatenate(locs[bb])
            seg_pos[j][bb] = np.concatenate(poss[bb])
            cnt[j][bb] = seg_loc[j][bb].shape[0]

    # shared program shape: per (slot, bank) column count = max over cores
    ncol = np.zeros((NSLOT, NBANK), dtype=np.int64)
    for s in range(NSLOT):
        for b in range(NBANK):
            cs = cnt[[s * 8 + c for c in range(N_CORES)], b]
            ncol[s][b] = -(-int(cs.max()) // 128)
    return {"groups": groups, "seg_loc": seg_loc, "seg_pos": seg_pos,
            "cnt": cnt, "ncol": ncol}


def _per_core_arrays(plan, we_hat16, core):
    """Build one core's iw / tokid / q arrays for both slots."""
    ncol = plan["ncol"]
    out = {}
    for s in range(NSLOT):
        j = s * 8 + core
        NC = int(ncol[s].sum())
        iw_segs = []
        tokid = np.full((NC, 128), -1.0, dtype=BF16)
        qpk = np.zeros((NC, 128, QE), dtype=BF16)
        c0 = 0
        for b in range(NBANK):
            nb = int(ncol[s][b])
            if nb == 0:
                continue
            npair = nb * 128
            loc = np.zeros(npair, dtype=np.int16)
            real = plan["seg_loc"][j][b]
            pos = plan["seg_pos"][j][b]
            n = real.shape[0]
            loc[:n] = real.astype(np.int16)
            iw_segs.append(_wrap16(loc))
            tk = tokid[c0 : c0 + nb].reshape(-1)
            tk[:n] = pos.astype(BF16)
            qp = qpk[c0 : c0 + nb].reshape(npair, QE)
            qp[:n] = we_hat16[plan["groups"][j][pos]]
            c0 += nb
        out[f"iw{s}"] = np.concatenate(iw_segs, axis=1)
        out[f"tokid{s}"] = np.ascontiguousarray(tokid.T)            # [128, NC]
        out[f"q{s}"] = np.ascontiguousarray(qpk.transpose(1, 0, 2))  # [128,NC,QE]
    return out


# --------------------------------------------------------------------------
# bass program (built per column-count signature)
# --------------------------------------------------------------------------

def _build_nc(ncol):
    nc = bacc.Bacc(None, target_bir_lowering=False)
    bf = mybir.dt.bfloat16
    f32 = mybir.dt.float32

    tbl_d = nc.dram_tensor("tbl", [V, TE], bf, kind="ExternalInput")
    # iota3[p, t, j] = t — materialized so the eq build needs no
    # innermost-stride-0 broadcast
    iota_d = nc.dram_tensor("iota", [128, 128, CHUNK], bf,
                            kind="ExternalInput")
    doc_d = nc.dram_tensor("doc", [NSLOT * 128, HD], f32,
                           kind="ExternalOutput")
    slot_in = []
    for s in range(NSLOT):
        NC = int(ncol[s].sum())
        slot_in.append({
            "q": nc.dram_tensor(f"q{s}", [128, NC, QE], bf,
                                kind="ExternalInput"),
            "tokid": nc.dram_tensor(f"tokid{s}", [128, NC], bf,
                                    kind="ExternalInput"),
            "iw": nc.dram_tensor(f"iw{s}", [128, 8 * NC], mybir.dt.int16,
                                 kind="ExternalInput"),
        })

    NC_MX = max(int(ncol[s].sum()) for s in range(NSLOT))

    with tile.TileContext(nc) as tc:
        with (
            tc.tile_pool(name="const", bufs=1) as const,
            tc.tile_pool(name="per_slot", bufs=2) as per_slot,
            tc.tile_pool(name="gpool", bufs=3) as gpool,
            tc.tile_pool(name="qpool", bufs=3) as qpool,
            tc.tile_pool(name="ppool", bufs=2) as ppool,
            tc.tile_pool(name="lpool", bufs=2) as lpool,
            tc.tile_pool(name="small", bufs=4) as small,
            tc.tile_pool(name="psum_o", bufs=2, space="PSUM") as psum_o_pool,
        ):
            iota_t = const.tile([128, 128, CHUNK], bf)

            for s in range(NSLOT):
                NC = int(ncol[s].sum())
                din = slot_in[s]

                iw_t = per_slot.tile([128, 8 * NC_MX], mybir.dt.int16,
                                     tag="iw_t")
                nc.sync.dma_start(out=iw_t[:, : 8 * NC], in_=din["iw"][:, :])
                tokid_t = per_slot.tile([128, NC_MX], bf, tag="tokid_t")
                nc.sync.dma_start(out=tokid_t[:, :NC], in_=din["tokid"][:, :])
                if s == 0:
                    nc.sync.dma_start(out=iota_t[:], in_=iota_d[:, :, :])

                psum_t = psum_o_pool.tile([128, RHS_W], f32, tag="psum_t",
                                          name=f"psum_{s}")

                # chunk list: (bank, global col, cols) within one bank each.
                # Taper the first chunks of slot 0 (fast pipeline warm-up)
                # and the last chunks of the final slot (short drain tail).
                chunks = []
                c0 = 0
                for b in range(NBANK):
                    nb = int(ncol[s][b])
                    sizes = []
                    rem = nb
                    if s == 0 and b == 0:
                        for t in (2, 2, 4, 8):
                            if rem >= t:
                                sizes.append(t)
                                rem -= t
                    tail = []
                    if s == NSLOT - 1 and b == NBANK - 1:
                        for t in (8, 4, 2, 2):
                            if rem >= t:
                                tail.append(t)
                                rem -= t
                    while rem > 0:
                        t = min(CHUNK, rem)
                        sizes.append(t)
                        rem -= t
                    sizes += tail
                    cc = 0
                    for t in sizes:
                        chunks.append((b, c0 + cc, t))
                        cc += t
                    c0 += nb
                last = len(chunks) - 1

                for ci, (b, gc, cols) in enumerate(chunks):
                    n = 128 * cols
                    g_t = gpool.tile([128, CHUNK, TE], bf, tag="g_t")
                    nc.gpsimd.dma_gather(
                        out_ap=g_t[:, :cols, :],
                        in_ap=tbl_d[b * BROWS : (b + 1) * BROWS, :],
                        idxs_ap=iw_t[:, 8 * gc : 8 * (gc + cols)],
                        num_idxs=n, num_idxs_reg=n, elem_size=TE,
                        single_packet=True,
                    )
                    q_t = qpool.tile([128, CHUNK, QE], bf, tag="q_t")
                    nc.sync.dma_start(out=q_t[:, :cols, :],
                                      in_=din["q"][:, gc : gc + cols, :])
                    prod = ppool.tile([128, CHUNK, QE], bf, tag="prod")
                    nc.vector.tensor_tensor(
                        out=prod[:, :cols, :], in0=g_t[:, :cols, 0:QE],
                        in1=q_t[:, :cols, :], op=mybir.AluOpType.mult,
                    )
                    scores = small.tile([128, CHUNK], f32, tag="scores")
                    nc.vector.tensor_reduce(
                        out=scores[:, :cols], in_=prod[:, :cols, :],
                        axis=mybir.AxisListType.X, op=mybir.AluOpType.add,
                    )
                    exps = small.tile([128, CHUNK], bf, tag="exps")
                    nc.scalar.activation(
                        out=exps[:, :cols], in_=scores[:, :cols],
                        func=mybir.ActivationFunctionType.Exp,
                    )
                    # lhsT_t[p, t, j] = (t == tokid[p, gc+j]) * exp[p, j]
                    eq_t = lpool.tile([128, 128, CHUNK], bf, tag="eq_t")
                    nc.vector.tensor_tensor(
                        out=eq_t[:, :, :cols], in0=iota_t[:, :, :cols],
                        in1=tokid_t[:, None, gc : gc + cols].to_broadcast(
                            [128, 128, cols]),
                        op=mybir.AluOpType.is_equal,
                    )
                    lhsT = lpool.tile([128, 128, CHUNK], bf, tag="lhsT")
                    nc.vector.tensor_tensor(
                        out=lhsT[:, :, :cols], in0=eq_t[:, :, :cols],
                        in1=exps[:, None, :cols].to_broadcast([128, 128, cols]),
                        op=mybir.AluOpType.mult,
                    )
                    for j in range(cols):
                        nc.tensor.matmul(
                            out=psum_t[:],
                            lhsT=lhsT[:, :, j],
                            rhs=g_t[:, j, ONE_POS : ONE_POS + RHS_W],
                            start=(ci == 0 and j == 0),
                            stop=(ci == last and j == cols - 1),
                        )

                recip = small.tile([128, 1], f32, tag="recip",
                                   name=f"recip_{s}")
                nc.vector.reciprocal(out=recip[:], in_=psum_t[:, 0:1])
                doc_sb = per_slot.tile([128, HD], f32, tag="doc_sb")
                nc.vector.tensor_scalar(
                    out=doc_sb[:], in0=psum_t[:, 1:RHS_W], scalar1=recip[:],
                    scalar2=None, op0=mybir.AluOpType.mult,
                )
                nc.sync.dma_start(out=doc_d[s * 128 : (s + 1) * 128, :],
                                  in_=doc_sb[:])

    nc.compile()
    return nc


# --------------------------------------------------------------------------
# entry point
# --------------------------------------------------------------------------

def kernel(word_embs, wordmem, bankmem, idx, lengths, _trace=False, **_kw):
    we = np.asarray(word_embs, dtype=np.float32)
    wm = np.asarray(wordmem, dtype=np.float32)
    bm = np.asarray(bankmem, dtype=np.float32)

    plan = _plan(idx, lengths)
    sig = tuple(int(x) for x in plan["ncol"].reshape(-1))
    if _CACHE.get("sig") != sig:
        _CACHE["nc"] = _build_nc(plan["ncol"])
        _CACHE["sig"] = sig
    nc = _CACHE["nc"]

    # combined bf16 table: [w_hat | 0-pad | 1.0 | bank], 512 elems = 1024B
    wnorm = np.sqrt((wm * wm).sum(axis=1, dtype=np.float32))
    wn = wm / np.maximum(wnorm, np.float32(1e-12))[:, None]
    tbl = np.zeros((V, TE), dtype=BF16)
    tbl[:, :WD] = wn.astype(BF16)
    tbl[:, ONE_POS] = np.float32(1.0)
    tbl[:, ONE_POS + 1 : ONE_POS + 1 + HD] = bm.astype(BF16)

    qnorm = np.sqrt((we * we).sum(axis=1, dtype=np.float32))
    we_hat = we / np.maximum(qnorm, np.float32(1e-12))[:, None]
    we_hat16 = np.zeros((NUM, QE), dtype=BF16)
    we_hat16[:, :WD] = we_hat.astype(BF16)

    iota = np.ascontiguousarray(np.broadcast_to(
        np.arange(128, dtype=np.float32)[None, :, None].astype(BF16),
        (128, 128, CHUNK)))

    in_maps = []
    for c in range(N_CORES):
        m = _per_core_arrays(plan, we_hat16, c)
        m["tbl"] = tbl
        m["iota"] = iota
        in_maps.append(m)

    kw = {"trace": True, "trace_cores": [0]} if _trace else {}
    res = run_bass_kernel_spmd(nc, in_maps, core_ids=list(range(N_CORES)), **kw)
    if _trace:
        print(f"HW exec time: {res.exec_time_ns} ns")
        _CACHE["last_trace"] = res

    out = np.zeros((NUM, HD), dtype=np.float32)
    for c in range(N_CORES):
        doc = res.results[c]["doc"]
        for s in range(NSLOT):
            out[plan["groups"][s * 8 + c]] = doc[s * 128 : (s + 1) * 128, :]
    return out
